# revision 16
# baseline (speedup 1.0000x reference)
"""CSNet forward on 8 Trainium2 NeuronCores (Bass/Tile).

Sharding: core = (batch b = core//2, H-half = core%2). Each core computes
output rows [half*256, half*256+256) of its batch element's [1, 512, 512]
output, using margin-recompute (slab of 288 rows = owned 256 + 16 margin each
side) so no halo exchange is needed. The only cross-core communication is a
[64,2] AllReduce per BatchNorm (10 total) for training-mode batch stats.

Layout: activations live in fp16. The persistent skip tensor h sits in SBUF as
[128, 146, 514]: partitions 0:64 = channels for slab rows [-16,130) ("lower
half", group index g = row+16), partitions 64:128 = channels for slab rows
[126,272) (row = g+126); 514 = 512 cols + 1 zero pad col each side. Conv3x3 is
9 accumulating matmuls (K=64 in-channels) packed 4-per-tap into the 128x128 PE
array via tile_position quadrants: col groups 0/64 = output rows g/g+1, row
groups 0/64 = lower/upper h halves, giving 4 output rows per PSUM round.
"""
import sys, os
for _p in ('/opt/trn_rl_repo', '/root/.axon_site/_ro/trn_rl_repo'):
    if os.path.isdir(_p) and _p not in sys.path:
        sys.path.insert(0, _p)

import dataclasses
import numpy as np
from contextlib import ExitStack

import concourse.bass as bass
import concourse.bacc as bacc
import concourse.mybir as mybir
from concourse.bass_utils import run_bass_kernel_spmd
from concourse.tile import TileContext
from concourse.alu_op_type import AluOpType

F32 = mybir.dt.float32
F16 = mybir.dt.float16
AF = mybir.ActivationFunctionType
AX = mybir.AxisListType

EPS = 1e-5
S, MARG, OWN, W = 288, 16, 256, 512
NG = 146          # groups (= rows per half; lower rows g-16, upper rows g+126)
WP = W + 2        # padded width in SBUF
UOFF = 142        # upper-half slab-index offset (slab idx = g + UOFF)
NGLOB = 4 * 512 * 512
# stats fold j-ranges (round j covers groups g=1+2j, g+2): [bank][half] -> (j0, j1)
# bankX rows (2j-15, 2j-14); bankY rows (2j+127, 2j+128); owned = [0,256)
JR = {(0, 0): (8, 72), (0, 1): (7, 71), (1, 0): (1, 65), (1, 1): (0, 64)}
BOUNDARY_LO = list(range(0, 16))       # groups whose lower row may be out-of-image
BOUNDARY_UP = list(range(130, 146))    # groups whose upper row may be out-of-image
N_STAGES = int(os.environ.get('CSNET_STAGES', '99'))
DEBUG_OUT = os.environ.get('CSNET_DEBUG', '') == '1'


def rap(ap, dims, offset):
    """Build a custom AP (list of [step, count]) on the same tensor."""
    return dataclasses.replace(ap, ap=[list(d) for d in dims], offset=offset)


# ---------------------------------------------------------------------------
# host-side input prep
# ---------------------------------------------------------------------------

def prep_inputs(x, params):
    x = np.asarray(x, np.float32)
    p = {k: np.asarray(v, np.float32) for k, v in params.items()}

    w1t = p['conv1_w'][:, :, 0, 0].T.copy()              # [64c, 256o]
    bc1 = np.tile(p['conv1_b'].reshape(2, 128), (1, 1))  # [2mh, 128]
    bc1 = np.ascontiguousarray(bc1.T)                    # [128, 2]

    w7 = p['blk1_w'][:, 0]                               # [64, 7, 7]
    w7t = np.zeros((49, 128), np.float32)
    w7t[:, 0:64] = w7.reshape(64, 49).T
    w7t[:, 64:128] = w7t[:, 0:64]

    layers = []
    for k in range(2, 7):
        layers += [('rb%d_c1' % k), ('rb%d_c2' % k)]
    layers += ['blk7']
    wt = np.zeros((11, 128, 9, 128), np.float32)
    biases = np.zeros((128, 13), np.float32)
    biases[:, 0] = np.tile(p['blk1_b'], 2)
    for li, name in enumerate(layers):
        wk = p[name + '_w']
        bk = p[name + '_b']
        # wk: [64o, 64c, 3, 3]
        for tap in range(9):
            dy, dx = tap // 3, tap % 3
            blkw = wk[:, :, dy, dx].T                    # [64c, 64o]
            wt[li, 0:64, tap, 0:64] = blkw
            wt[li, 0:64, tap, 64:128] = blkw
            wt[li, 64:128, tap, 0:64] = blkw
            wt[li, 64:128, tap, 64:128] = blkw
        biases[:, 1 + li] = np.tile(bk, 2)
    w8 = p['blk8_w'][0]                                  # [64, 3, 3]
    w8t = np.zeros((128, 9, 128), np.float32)
    for tap in range(9):
        dy, dx = tap // 3, tap % 3
        for j in range(4):
            w8t[0:64, tap, 32 * j] = w8[:, dy, dx]
            w8t[64:128, tap, 32 * j] = w8[:, dy, dx]
    biases[:, 12] = p['blk8_b'][0]

    gb = np.zeros((128, 10, 2), np.float32)
    for k in range(2, 7):
        for ci, c in enumerate(('1', '2')):
            bn = 2 * (k - 2) + ci
            gb[:, bn, 0] = np.tile(p['rb%d_bn%s_g' % (k, c)], 2)
            gb[:, bn, 1] = np.tile(p['rb%d_bn%s_b' % (k, c)], 2)

    alphas = np.zeros((128, 7), np.float32)
    alphas[:, 0] = float(p['blk1_alpha'])
    for k in range(2, 7):
        alphas[:, k - 1] = float(p['rb%d_alpha' % k])
    alphas[:, 6] = float(p['blk7_alpha'])

    common = {
        'w1t': w1t.astype(np.float16), 'w7t': w7t.astype(np.float16),
        'wt': wt.astype(np.float16), 'w8t': w8t.astype(np.float16),
        'bias': biases, 'bc1': bc1, 'gb': gb, 'alpha': alphas,
        'ident': np.eye(128, dtype=np.float16),
    }

    in_maps = []
    for core in range(8):
        b, half = core // 2, core % 2
        xb = x[b]
        z = np.zeros((64, 1, 32), np.float32)
        if half == 0:
            sh = np.concatenate([z, xb[:, 0:17, :]], axis=1)
        else:
            sh = np.concatenate([xb[:, 15:32, :], z], axis=1)
        zm = np.ones(S, np.float32)
        if half == 0:
            zm[:MARG] = 0.0
        else:
            zm[S - MARG:] = 0.0
        zmg = np.zeros((128, NG), np.float32)
        for g in range(NG):
            zmg[0:64, g] = zm[g]            # lower row g-16 -> slab idx g
            zmg[64:128, g] = zm[g + UOFF]   # upper row g+126 -> slab idx g+142
        m = dict(common)
        m['xs'] = np.ascontiguousarray(sh.reshape(64, 18 * 32)).astype(np.float16)
        m['zmg'] = zmg
        in_maps.append(m)
    return in_maps


# ---------------------------------------------------------------------------
# program builder
# ---------------------------------------------------------------------------

def build_nc():
    nc = bacc.Bacc("TRN2", target_bir_lowering=False, num_devices=8)

    xs = nc.declare_dram_parameter("xs", [64, 18 * 32], F16, isOutput=False)
    zmg = nc.declare_dram_parameter("zmg", [128, NG], F32, isOutput=False)
    w1t = nc.declare_dram_parameter("w1t", [64, 256], F16, isOutput=False)
    w7t = nc.declare_dram_parameter("w7t", [49, 128], F16, isOutput=False)
    wt = nc.declare_dram_parameter("wt", [11, 128, 9, 128], F16, isOutput=False)
    w8t = nc.declare_dram_parameter("w8t", [128, 9, 128], F16, isOutput=False)
    biasP = nc.declare_dram_parameter("bias", [128, 13], F32, isOutput=False)
    bc1P = nc.declare_dram_parameter("bc1", [128, 2], F32, isOutput=False)
    gbP = nc.declare_dram_parameter("gb", [128, 10, 2], F32, isOutput=False)
    alphaP = nc.declare_dram_parameter("alpha", [128, 7], F32, isOutput=False)
    idP = nc.declare_dram_parameter("ident", [128, 128], F16, isOutput=False)
    out = nc.declare_dram_parameter("out", [256, 512], F32, isOutput=True)

    img = nc.dram_tensor("img", [294, 518], F16)
    rA = nc.dram_tensor("rA", [128, NG, 512], F16)
    rB = nc.dram_tensor("rB", [128, NG, 512], F16)
    b1d = nc.dram_tensor("b1d", [128, NG, 512], F16)
    ccin = [nc.dram_tensor("ccin%d" % i, [64, 2], F32) for i in range(10)]
    ccout = [nc.dram_tensor("ccout%d" % i, [64, 2], F32, addr_space="Shared")
             for i in range(10)]
    dbg = {}
    if DEBUG_OUT:
        for nm in ('dh', 'dra', 'drb'):
            dbg[nm] = nc.declare_dram_parameter(nm, [128, NG, 512], F32, isOutput=True)

    with TileContext(nc) as tc, ExitStack() as ctx:
        cpool = ctx.enter_context(tc.tile_pool(name="consts", bufs=1))
        hpool = ctx.enter_context(tc.tile_pool(name="h", bufs=1))
        wpool = ctx.enter_context(tc.tile_pool(name="w", bufs=2))
        evp = ctx.enter_context(tc.tile_pool(name="ev", bufs=6))
        evf32p = ctx.enter_context(tc.tile_pool(name="evf32", bufs=2))
        statsp = ctx.enter_context(tc.tile_pool(name="stats", bufs=2))
        smallp = ctx.enter_context(tc.tile_pool(name="small", bufs=4))
        psum = ctx.enter_context(tc.tile_pool(name="ps", bufs=4, space="PSUM"))
        rbufp = None  # created after blk1 (shares SBUF with the imcol pool)

        # ---- persistent constants
        zm_sb = cpool.tile([128, NG], F32)
        nc.sync.dma_start(zm_sb[:, :], zmg[:, :])
        bias_sb = cpool.tile([128, 13], F32)
        nc.sync.dma_start(bias_sb[:, :], biasP[:, :])
        bc1_sb = cpool.tile([128, 2], F32)
        nc.sync.dma_start(bc1_sb[:, :], bc1P[:, :])
        gb_sb = cpool.tile([128, 10, 2], F32)
        nc.sync.dma_start(gb_sb[:, :, :], gbP[:, :, :])
        al_sb = cpool.tile([128, 7], F32)
        nc.sync.dma_start(al_sb[:, :], alphaP[:, :])
        st_sb = cpool.tile([128, 20], F32)      # (s,t) per BN
        w8_sb = cpool.tile([128, 9, 128], F16)
        nc.sync.dma_start(w8_sb[:, :, :], w8t[:, :, :])

        h = hpool.tile([128, NG, WP], F16)
        nc.vector.memset(h[:, :, 0:1], 0.0)
        nc.vector.memset(h[:, :, WP - 1:WP], 0.0)

        # =================================================================
        # stage 0: conv1 (1x1, 64->256) + depth_to_space -> img in DRAM
        # =================================================================
        setupp = tc.tile_pool(name="setup", bufs=1)
        setup_pool = setupp.__enter__()
        xs_sb = setup_pool.tile([64, 18 * 32], F16)
        nc.sync.dma_start(xs_sb[:, :], xs[:, :])
        w1_sb = setup_pool.tile([64, 256], F16)
        nc.sync.dma_start(w1_sb[:, :], w1t[:, :])

        # zero-pad rows/cols of img
        zz = setup_pool.tile([128, 1024], F16)
        nc.vector.memset(zz[:, :], 0.0)
        iap = img[0, :]   # base AP on img
        nc.sync.dma_start(rap(iap, [[518, 3], [1, 518]], 0), zz[0:3, 0:518])
        nc.sync.dma_start(rap(iap, [[518, 3], [1, 518]], 291 * 518), zz[0:3, 0:518])
        nc.sync.dma_start(rap(iap, [[518, 288], [1, 3]], 3 * 518), zz[0:1, 0:864])
        nc.sync.dma_start(rap(iap, [[518, 288], [1, 3]], 3 * 518 + 515), zz[0:1, 0:864])

        id_sb = setup_pool.tile([128, 128], F16)
        nc.sync.dma_start(id_sb[:, :], idP[:, :])
        for mh in range(2):
            for nh in range(2):
                ps = psum.tile([128, 512], F32, tag="ps")
                nc.tensor.matmul(ps[:, 0:288], w1_sb[:, mh * 128:(mh + 1) * 128],
                                 xs_sb[:, nh * 288:(nh + 1) * 288],
                                 start=True, stop=True)
                ev = evp.tile([128, 512], F16, tag="ev")
                nc.scalar.activation(ev[:, 0:288], ps[:, 0:288], AF.Identity,
                                     bias=bc1_sb[:, mh:mh + 1])
                # transpose so (k1,k2) moves to the free dim, then scatter.
                # ev[p=(k1,k2), f=(ii,j)] -> tp[f=(iic,j), p128=(k1,k2)]
                for ic in range(3):          # ii chunks: 4,4,1
                    nii = 4 if ic < 2 else 1
                    tp = psum.tile([128, 512], F16, tag="ps")
                    nc.tensor.transpose(tp[0:32 * nii, 0:128],
                                        ev[:, 128 * ic:128 * ic + 32 * nii],
                                        id_sb[:, :])
                    tt = evp.tile([128, 512], F16, tag="ev")
                    nc.vector.tensor_copy(tt[0:32 * nii, 0:128], tp[0:32 * nii, 0:128])
                    tap_ = tt[:, :]
                    tpst = tap_.ap[0][0]
                    for iic in range(nii):
                        ii = 4 * ic + iic
                        off = (3 + 144 * nh + 8 * mh + 16 * ii) * 518 + 3
                        dst = rap(iap, [[16, 32], [518, 8], [1, 16]], off)
                        ssrc = rap(tap_, [[tpst, 32], [16, 8], [1, 16]],
                                   tap_.offset + 32 * iic * tpst)
                        nc.sync.dma_start(dst, ssrc)

        setupp.__exit__(None, None, None)

        # =================================================================
        # stage 1: blk1 7x7 conv (1 -> 64 ch) + prelu -> h (and b1d copy)
        # =================================================================
        w7_sb = cpool.tile([49, 128], F16)
        nc.sync.dma_start(w7_sb[:, :], w7t[:, :])
        with tc.tile_pool(name="imcol", bufs=2) as imcolp:
            for i0 in range(0, NG, 3):
                n = min(3, NG - i0)
                im = imcolp.tile([49, 2, 3, 512], F16)
                for dy in range(7):
                    for hf in range(2):
                        s2 = rap(iap, [[1, 7], [518, n], [1, 512]],
                                 (i0 + dy + UOFF * hf) * 518)
                        nc.sync.dma_start(im[7 * dy:7 * dy + 7, hf, 0:n, :], s2)
                for gg in range(n):
                    g = i0 + gg
                    ps = psum.tile([128, 512], F32, tag="ps")
                    nc.tensor.matmul(ps[0:64, :], w7_sb[:, 0:64], im[:, 0, gg, :],
                                     tile_position=(0, 0), start=True, stop=True)
                    nc.tensor.matmul(ps[64:128, :], w7_sb[:, 64:128], im[:, 1, gg, :],
                                     tile_position=(0, 64), start=True, stop=True)
                    nc.scalar.activation(h[:, g, 1:513], ps[:, :], AF.Prelu,
                                         bias=bias_sb[:, 0:1], scale=1.0,
                                         alpha=al_sb[:, 0:1])
                    if g in BOUNDARY_LO or g in BOUNDARY_UP:
                        nc.vector.tensor_scalar(h[:, g, 1:513], h[:, g, 1:513],
                                                zm_sb[:, g:g + 1], None, AluOpType.mult)
                    nc.sync.dma_start(b1d[:, g, :], h[:, g, 1:513])

        rbufp = ctx.enter_context(tc.tile_pool(name="rbuf", bufs=2))

        # =================================================================
        # helpers for the 3x3 passes
        # =================================================================
        def h_update(rsrc, bn):
            """h += s*r + t (in fp16), then re-zero out-of-image rows."""
            sA = st_sb[:, 2 * bn:2 * bn + 1]
            tA = st_sb[:, 2 * bn + 1:2 * bn + 2]
            for i0 in range(0, NG, 8):
                n = min(8, NG - i0)
                rb = rbufp.tile([128, 8, WP], F16, tag="rb")
                nc.sync.dma_start(rb[:, 0:n, 1:513], rsrc[:, i0:i0 + n, :])
                nc.vector.tensor_scalar(rb[:, 0:n, 1:513], rb[:, 0:n, 1:513],
                                        sA, tA, AluOpType.mult, AluOpType.add)
                nc.vector.tensor_tensor(h[:, i0:i0 + n, 1:513], h[:, i0:i0 + n, 1:513],
                                        rb[:, 0:n, 1:513], AluOpType.add)
            for g in BOUNDARY_LO + BOUNDARY_UP:
                nc.vector.tensor_scalar(h[:, g, 1:513], h[:, g, 1:513],
                                        zm_sb[:, g:g + 1], None, AluOpType.mult)

        def conv3x3(wl, rdst, bias_col, rhs_lo, rhs_up, stats, prelu_alpha_col=None):
            """72 rounds of 2 groups; evac (+optional prelu) -> rdst; bn_stats."""
            wtile = wpool.tile([128, 9, 128], F16)
            nc.sync.dma_start(wtile[:, :, :], wt[wl, :, :, :])
            for j in range(72):
                g = 1 + 2 * j
                psX = psum.tile([128, 512], F32, tag="ps")
                psY = psum.tile([128, 512], F32, tag="ps")
                for tap in range(9):
                    dy, dx = tap // 3 - 1, tap % 3 - 1
                    st = dict(start=(tap == 0), stop=(tap == 8))
                    nc.tensor.matmul(psX[0:64, :], wtile[0:64, tap, 0:64],
                                     rhs_lo(g + dy, dx), tile_position=(0, 0), **st)
                    nc.tensor.matmul(psX[64:128, :], wtile[0:64, tap, 64:128],
                                     rhs_lo(g + 1 + dy, dx), tile_position=(0, 64), **st)
                    nc.tensor.matmul(psY[0:64, :], wtile[64:128, tap, 0:64],
                                     rhs_up(g + dy, dx), tile_position=(64, 0), **st)
                    nc.tensor.matmul(psY[64:128, :], wtile[64:128, tap, 64:128],
                                     rhs_up(g + 1 + dy, dx), tile_position=(64, 64), **st)
                for bank, ps in ((0, psX), (1, psY)):
                    ev = evp.tile([128, 512], F16)
                    if prelu_alpha_col is None:
                        nc.scalar.activation(ev[:, :], ps[:, :], AF.Identity,
                                             bias=bias_sb[:, bias_col:bias_col + 1])
                    else:
                        nc.scalar.activation(ev[:, :], ps[:, :], AF.Prelu,
                                             bias=bias_sb[:, bias_col:bias_col + 1],
                                             scale=1.0,
                                             alpha=al_sb[:, prelu_alpha_col:prelu_alpha_col + 1])
                    if stats is not None:
                        nc.vector.bn_stats(stats[:, j, bank, :], ev[:, :])
                    # partitions 0:64 -> row g, 64:128 -> row g+1 of this bank's half
                    base = (bank * 64 * NG + g) * 512
                    dst = rap(rdst[0, 0, :], [[512, 2], [NG * 512, 64], [1, 512]], base)
                    nc.sync.dma_start(dst, ev[:, :])
                    # duplicated-row fixups: lower slot 145 (= upper row 129,
                    # bank Y of round g=3), upper slot 0 (= lower row 126,
                    # bank X partitions 64: of round g=141)
                    if bank == 1 and g == 3:
                        nc.sync.dma_start(rdst[0:64, 145, :], ev[0:64, :])
                    if bank == 0 and g == 141:
                        nc.sync.dma_start(rdst[64:128, 0, :], ev[64:128, :])

        def fold_and_allreduce(stats, bn):
            """stats [128,72,2,6] -> global (mean -> s,t) via AllReduce."""
            fs = smallp.tile([128, 8], F32)
            sq = smallp.tile([128, 160], F32)
            stf = stats[:, :, :, :]
            pst = stf.ap[0][0]
            for half, hp in ((0, 0), (1, 64)):
                for bank in range(2):
                    j0, j1 = JR[(bank, half)]
                    nj = j1 - j0
                    mean_ap = rap(stf, [[pst, 64], [12, nj], [3, 2]],
                                  stf.offset + hp * pst + j0 * 12 + bank * 6 + 1)
                    m2_ap = rap(stf, [[pst, 64], [12, nj], [3, 2]],
                                stf.offset + hp * pst + j0 * 12 + bank * 6 + 2)
                    nc.vector.reduce_sum(fs[hp:hp + 64, bank:bank + 1], mean_ap, axis=AX.XY)
                    nc.vector.reduce_sum(fs[hp:hp + 64, 2 + bank:3 + bank], m2_ap, axis=AX.XY)
                    # sum of mean^2: square into scratch then reduce
                    sqa = sq[hp:hp + 64, 0:2 * nj]
                    nc.scalar.activation(sqa, mean_ap, AF.Square)
                    nc.vector.reduce_sum(fs[hp:hp + 64, 4 + bank:5 + bank], sqa, axis=AX.XY)
            # fs cols: 0,1 = sum-mean (bankX, bankY); 2,3 = sum-M2; 4,5 = sum-mean^2
            acc = smallp.tile([128, 4], F32)
            nc.vector.tensor_tensor(acc[:, 0:1], fs[:, 0:1], fs[:, 1:2], AluOpType.add)
            nc.vector.tensor_tensor(acc[:, 1:2], fs[:, 2:3], fs[:, 3:4], AluOpType.add)
            nc.vector.tensor_tensor(acc[:, 2:3], fs[:, 4:5], fs[:, 5:6], AluOpType.add)
            pk = smallp.tile([128, 2], F32)
            nc.vector.tensor_scalar(pk[:, 0:1], acc[:, 0:1], 256.0, None, AluOpType.mult)
            nc.vector.scalar_tensor_tensor(pk[:, 1:2], acc[:, 2:3], 256.0, acc[:, 1:2],
                                           AluOpType.mult, AluOpType.add)
            # fold partition halves
            pk2 = smallp.tile([128, 2], F32)
            nc.sync.dma_start(pk2[0:64, :], pk[64:128, :])
            loc = smallp.tile([64, 2], F32)
            nc.vector.tensor_tensor(loc[:, :], pk[0:64, :], pk2[0:64, :], AluOpType.add)
            nc.sync.dma_start(ccin[bn][:, :], loc[:, :])
            nc.gpsimd.collective_compute(
                "AllReduce", AluOpType.add, replica_groups=[list(range(8))],
                ins=[ccin[bn][:, :]], outs=[ccout[bn][:, :]])
            gl = smallp.tile([64, 6], F32)
            nc.sync.dma_start(gl[:, 0:2], ccout[bn][:, :])
            # mean, ex2, var, rstd
            nc.vector.tensor_scalar(gl[:, 2:3], gl[:, 0:1], 1.0 / NGLOB, None, AluOpType.mult)
            nc.vector.tensor_scalar(gl[:, 3:4], gl[:, 1:2], 1.0 / NGLOB, None, AluOpType.mult)
            msq = smallp.tile([64, 2], F32)
            nc.vector.tensor_scalar(msq[:, 0:1], gl[:, 2:3], gl[:, 2:3], None, AluOpType.mult)
            nc.vector.tensor_scalar(gl[:, 4:5], msq[:, 0:1], -1.0, gl[:, 3:4],
                                    AluOpType.mult, AluOpType.add)     # var
            nc.vector.tensor_scalar(gl[:, 5:6], gl[:, 4:5], EPS, None, AluOpType.add)
            sqv = smallp.tile([64, 1], F32)
            nc.scalar.activation(sqv[:, :], gl[:, 5:6], AF.Sqrt)
            rstd = smallp.tile([64, 1], F32)
            nc.vector.reciprocal(rstd[:, :], sqv[:, :])
            nc.vector.tensor_scalar(st_sb[0:64, 2 * bn:2 * bn + 1], rstd[:, :],
                                    gb_sb[0:64, bn, 0:1], None, AluOpType.mult)  # s
            ms = smallp.tile([64, 1], F32)
            nc.vector.tensor_scalar(ms[:, :], gl[:, 2:3],
                                    st_sb[0:64, 2 * bn:2 * bn + 1], None, AluOpType.mult)
            nc.vector.tensor_scalar(st_sb[0:64, 2 * bn + 1:2 * bn + 2], ms[:, :],
                                    -1.0, gb_sb[0:64, bn, 1:2],
                                    AluOpType.mult, AluOpType.add)               # t
            nc.sync.dma_start(st_sb[64:128, 2 * bn:2 * bn + 2],
                              st_sb[0:64, 2 * bn:2 * bn + 2])

        def rhs_h(half):
            def f(gi, dx):
                p0 = 64 * half
                return h[p0:p0 + 64, gi, 1 + dx:513 + dx]
            return f

        def make_xin(rsrc, bn, alpha_col, i0, n):
            """load rows i0-1 .. i0+n+1, prelu(s*r+t) in place -> conv input."""
            xin = rbufp.tile([128, 8, WP], F16, tag="rb")
            nc.vector.memset(xin[:, :, 0:1], 0.0)
            nc.vector.memset(xin[:, :, WP - 1:WP], 0.0)
            nc.sync.dma_start(xin[:, 0:n + 2, 1:513], rsrc[:, i0 - 1:i0 + n + 1, :])
            nc.scalar.activation(xin[:, 0:n + 2, 1:513], xin[:, 0:n + 2, 1:513],
                                 AF.Prelu,
                                 bias=st_sb[:, 2 * bn + 1:2 * bn + 2],
                                 scale=st_sb[:, 2 * bn:2 * bn + 1],
                                 alpha=al_sb[:, alpha_col:alpha_col + 1])
            for g in BOUNDARY_LO + BOUNDARY_UP:
                if i0 - 1 <= g < i0 + n + 1:
                    nc.vector.tensor_scalar(xin[:, g - i0 + 1, 1:513],
                                            xin[:, g - i0 + 1, 1:513],
                                            zm_sb[:, g:g + 1], None, AluOpType.mult)
            return xin

        # =================================================================
        # residual blocks
        # =================================================================
        stage = 2
        rbufs = (rA, rB)
        for k in range(2, 7):
            if N_STAGES < stage:
                break
            li1, li2 = 2 * (k - 2), 2 * (k - 2) + 1
            bn1, bn2 = li1, li2
            # ---- pass A: r1 = conv(h)+b ; stats -> s1,t1
            if k > 2:
                h_update(rB, bn1 - 1)
            stats = statsp.tile([128, 72, 2, 6], F32, tag='st')
            conv3x3(li1, rA, 1 + li1, rhs_h(0), rhs_h(1), stats)
            fold_and_allreduce(stats, bn1)
            # ---- pass B: r2 = conv(prelu(s1*r1+t1))+b ; stats -> s2,t2
            stats2 = statsp.tile([128, 72, 2, 6], F32, tag='st')
            wtile = wpool.tile([128, 9, 128], F16)
            nc.sync.dma_start(wtile[:, :, :], wt[li2, :, :, :])
            for i0 in range(1, 145, 6):
                n = min(6, 145 - i0)
                xin = make_xin(rA, bn1, k - 1, i0, n)
                for j0 in range(0, n, 2):
                    g = i0 + j0
                    jj = (g - 1) // 2
                    psX = psum.tile([128, 512], F32, tag="ps")
                    psY = psum.tile([128, 512], F32, tag="ps")
                    for tap in range(9):
                        dy, dx = tap // 3 - 1, tap % 3 - 1
                        st = dict(start=(tap == 0), stop=(tap == 8))
                        def xr(gi, half):
                            p0 = 64 * half
                            return xin[p0:p0 + 64, gi - i0 + 1, 1 + dx:513 + dx]
                        nc.tensor.matmul(psX[0:64, :], wtile[0:64, tap, 0:64],
                                         xr(g + dy, 0), tile_position=(0, 0), **st)
                        nc.tensor.matmul(psX[64:128, :], wtile[0:64, tap, 64:128],
                                         xr(g + 1 + dy, 0), tile_position=(0, 64), **st)
                        nc.tensor.matmul(psY[0:64, :], wtile[64:128, tap, 0:64],
                                         xr(g + dy, 1), tile_position=(64, 0), **st)
                        nc.tensor.matmul(psY[64:128, :], wtile[64:128, tap, 64:128],
                                         xr(g + 1 + dy, 1), tile_position=(64, 64), **st)
                    for bank, ps in ((0, psX), (1, psY)):
                        ev = evp.tile([128, 512], F16)
                        nc.scalar.activation(ev[:, :], ps[:, :], AF.Identity,
                                             bias=bias_sb[:, 1 + li2:2 + li2])
                        nc.vector.bn_stats(stats2[:, jj, bank, :], ev[:, :])
                        base = (bank * 64 * NG + g) * 512
                        dst = rap(rB[0, 0, :], [[512, 2], [NG * 512, 64], [1, 512]], base)
                        nc.sync.dma_start(dst, ev[:, :])
                        if bank == 1 and g == 3:
                            nc.sync.dma_start(rB[0:64, 145, :], ev[0:64, :])
                        if bank == 0 and g == 141:
                            nc.sync.dma_start(rB[64:128, 0, :], ev[64:128, :])
            fold_and_allreduce(stats2, bn2)
            stage += 1

        # =================================================================
        # blk7: h6 = h + affine(r2_rb6); b7 = prelu(conv(h6)+b) -> rA
        # =================================================================
        if N_STAGES >= 7:
            h_update(rB, 9)
            conv3x3(10, rA, 11, rhs_h(0), rhs_h(1), None, prelu_alpha_col=6)

        # =================================================================
        # blk8: out = conv3x3(b1 + b7) [64 -> 1], owned rows only
        # =================================================================
        if N_STAGES >= 8:
            for c in range(32):
                loL, loU = 16 + 4 * c, 2 + 4 * c
                xin = rbufp.tile([128, 8, WP], F16, tag="rb")
                nc.vector.memset(xin[:, :, 0:1], 0.0)
                nc.vector.memset(xin[:, :, WP - 1:WP], 0.0)
                nc.sync.dma_start(xin[0:64, 0:6, 1:513], b1d[0:64, loL - 1:loL + 5, :])
                nc.sync.dma_start(xin[64:128, 0:6, 1:513], b1d[64:128, loU - 1:loU + 5, :])
                rb7 = rbufp.tile([128, 8, WP], F16, tag="rb")
                nc.sync.dma_start(rb7[0:64, 0:6, 1:513], rA[0:64, loL - 1:loL + 5, :])
                nc.sync.dma_start(rb7[64:128, 0:6, 1:513], rA[64:128, loU - 1:loU + 5, :])
                nc.vector.tensor_tensor(xin[:, 0:6, 1:513], xin[:, 0:6, 1:513],
                                        rb7[:, 0:6, 1:513], AluOpType.add)
                if c == 0:
                    nc.vector.tensor_scalar(xin[0:64, 0, 1:513], xin[0:64, 0, 1:513],
                                            zm_sb[0:64, 15:16], None, AluOpType.mult)
                if c == 31:
                    nc.vector.tensor_scalar(xin[64:128, 5, 1:513], xin[64:128, 5, 1:513],
                                            zm_sb[64:128, 130:131], None, AluOpType.mult)
                for q in range(1):
                    gL = loL
                    psX = psum.tile([128, 512], F32, tag="ps")
                    psY = psum.tile([128, 512], F32, tag="ps")
                    for tap in range(9):
                        dy, dx = tap // 3 - 1, tap % 3 - 1
                        st = dict(start=(tap == 0), stop=(tap == 8))
                        for jj in range(4):
                            iL = gL + jj - loL + 1
                            nc.tensor.matmul(psX[32 * jj:32 * jj + 32, :],
                                             w8_sb[0:64, tap, 32 * jj:32 * jj + 32],
                                             xin[0:64, iL + dy, 1 + dx:513 + dx],
                                             tile_position=(0, 32 * jj), **st)
                            nc.tensor.matmul(psY[32 * jj:32 * jj + 32, :],
                                             w8_sb[64:128, tap, 32 * jj:32 * jj + 32],
                                             xin[64:128, iL + dy, 1 + dx:513 + dx],
                                             tile_position=(64, 32 * jj), **st)
                    for bank, ps in ((0, psX), (1, psY)):
                        evf = evf32p.tile([128, 512], F32)
                        nc.scalar.activation(evf[:, :], ps[:, :], AF.Identity,
                                             bias=bias_sb[:, 12:13])
                        r0 = 4 * c + (128 if bank else 0)
                        src = rap(evf[:, :], [[evf[:, :].ap[0][0] * 32, 4], [1, 512]],
                                  evf[:, :].offset)
                        nc.sync.dma_start(out[r0:r0 + 4, :], src)

        if DEBUG_OUT:
            hf = smallp.tile([128, 2, 512], F32, tag="dbgh")
            for g in range(NG):
                nc.vector.tensor_copy(hf[:, 0, :], h[:, g, 1:513])
                nc.sync.dma_start(dbg['dh'][:, g, :], hf[:, 0, :])
            for nm, rt in (('dra', rA), ('drb', rB)):
                for g in range(NG):
                    rbt = rbufp.tile([128, 8, WP], F16, tag="rb")
                    nc.sync.dma_start(rbt[:, 0:1, 1:513], rt[:, g:g + 1, :])
                    nc.vector.tensor_copy(hf[:, 1, :], rbt[:, 0, 1:513])
                    nc.sync.dma_start(dbg[nm][:, g, :], hf[:, 1, :])

    nc.finalize()
    return nc


def _final_dummy(nc, out, evp):
    pass


_NC_CACHE = {}


def kernel(x, params):
    in_maps = prep_inputs(x, params)
    key = 'nc'
    if key not in _NC_CACHE:
        _NC_CACHE[key] = build_nc()
    nc = _NC_CACHE[key]
    res = run_bass_kernel_spmd(nc, in_maps, list(range(8)))
    out = np.zeros((4, 1, 512, 512), np.float32)
    for core in range(8):
        b, half = core // 2, core % 2
        out[b, 0, half * 256:(half + 1) * 256, :] = res.results[core]['out']
    return out
